# revision 6
# baseline (speedup 1.0000x reference)
"""Trainium2 Bass kernel for nn_NeuralNet_19250043421419.

Row-normalize x (mean/std over D=3072, ddof=1) then a 3-layer MLP
(3072->32->32->10) with LeakyReLU(0.01) after every layer.

Strategy: pure data parallel over 8 NeuronCores (batch 32768 -> 4096/core).
Per core, rows are processed in blocks with graduated sizes (a 128-row
opener, 512-row steady state, 256/128 closers) so the first block's
compute chain is exposed for only ~1 load unit and the last block's
post-load tail is short.  The DMA stream itself is the same 32 x
[128, 3072] cast-loads in row order (the SWDGE queue drains them FIFO
at line rate), so the load phase is unaffected by the grouping.

Key scheduling details:
  - idh/idf identities are DMAd FIRST (and w1t as a flat AP) so the PE
    transposes are not gated on const arrival (~25us in the old order).
  - ACT warm-up ops right after the const loads force both activation
    table loads off the critical path.
  - The output is stored transposed ([10, B_CORE]) straight from the
    layer-3 activation tile and transposed on the host; this removes
    the PE out-transposes, the DVE out-copy, and a PSUM tile, and makes
    the store descriptors contiguous.

Per block of N rows (ns = N/128 sub-tiles):
  - DMA x in natural layout, casting fp32->fp16 in the SWDGE DMA.
  - bn_stats/bn_aggr on DVE for per-row mean/var, aggregated into one
    [128, 2, NSUB] tile; one ACT rsqrt per block covers all sub-tiles.
  - PE transposes mean/inv columns into row vectors; one fp16 ACT copy
    extracts both rows.
  - PE transposes x into [d, i] tiles (fp16 matmuls vs identity), ACT
    copies PSUM->SBUF casting to fp16, and PE streams the transposed
    tiles against w1^T accumulating y0_raw in PSUM over 24 K-chunks
    (double-buffered in 1024-column PSUM groups).
  - Normalization is folded in afterwards: (x-m)/s @ w1^T =
    (y0_raw - m * rowsum(w1)) / s.  The mean-correction is a K=1 fp16
    matmul into the same PSUM group; the 1/s scaling is a DVE multiply
    against a partition-broadcast fp16 row vector.
  - Layers 2/3 are small matmuls in the transposed layout where the
    biases are per-partition ACT Lrelu bias APs.
"""
import os
import sys

for _p in ("/opt/trn_rl_repo", "/root/.axon_site/_ro/trn_rl_repo"):
    if os.path.isdir(_p) and _p not in sys.path:
        sys.path.append(_p)

import numpy as np

import concourse.bass as bass
import concourse.bacc as bacc
import concourse.tile as tile
from concourse import mybir
from concourse.bass_utils import run_bass_kernel_spmd

F32 = mybir.dt.float32
F16 = mybir.dt.float16
AF = mybir.ActivationFunctionType

N_CORES = 8
B = 32768
D = 3072
H = 32
O = 10
B_CORE = B // N_CORES      # 4096
IBLK = 512                 # max rows per block
NSUB = IBLK // 128         # 4 sub-tiles of 128 rows max
NCHUNK = D // 128          # 24 contraction chunks
DDOF_SCALE = float(D) / float(D - 1)

# Graduated block sizes: short exposed chain at start and end, 512-row
# steady state in the middle.  Sums to B_CORE.
BLOCKS = [128] + [512] * 7 + [256, 128]
assert sum(BLOCKS) == B_CORE

LAST_EXEC_NS = None
_CACHE = {}


def _build():
    nc = bacc.Bacc("TRN2", target_bir_lowering=False, debug=False, num_devices=1)

    x_d = nc.dram_tensor("x", [B_CORE, D], F32, kind="ExternalInput").ap()
    w1t_d = nc.dram_tensor("w1t", [128, NCHUNK * H], F16, kind="ExternalInput").ap()
    w2t_d = nc.dram_tensor("w2t", [H, H], F16, kind="ExternalInput").ap()
    w3t_d = nc.dram_tensor("w3t", [H, O], F16, kind="ExternalInput").ap()
    negs_d = nc.dram_tensor("negs", [1, H], F16, kind="ExternalInput").ap()
    b1_d = nc.dram_tensor("b1c", [H, 1], F32, kind="ExternalInput").ap()
    b2_d = nc.dram_tensor("b2c", [H, 1], F32, kind="ExternalInput").ap()
    b3_d = nc.dram_tensor("b3c", [O, 1], F32, kind="ExternalInput").ap()
    idh_d = nc.dram_tensor("idh", [128, 128], F16, kind="ExternalInput").ap()
    idf_d = nc.dram_tensor("idf", [128, 128], F32, kind="ExternalInput").ap()
    y_d = nc.dram_tensor("y", [O, B_CORE], F32, kind="ExternalOutput").ap()

    with tile.TileContext(nc) as tc:
        with tc.tile_pool(name="consts", bufs=1) as consts, \
             tc.tile_pool(name="xpool", bufs=14) as xpool, \
             tc.tile_pool(name="xtpool", bufs=4) as xtpool, \
             tc.tile_pool(name="spool", bufs=4) as spool, \
             tc.tile_pool(name="pxt", bufs=2, space="PSUM") as pxt_pool, \
             tc.tile_pool(name="py0", bufs=2, space="PSUM") as py0_pool, \
             tc.tile_pool(name="pl", bufs=2, space="PSUM") as pl_pool:

            # ---- constants (identities first: the PE transposes gate on
            # them; w1t flat so its descriptors stay contiguous) ----
            idh_sb = consts.tile([128, 128], F16)
            nc.sync.dma_start(out=idh_sb, in_=idh_d)
            idf_sb = consts.tile([128, 128], F32)
            nc.sync.dma_start(out=idf_sb, in_=idf_d)
            w1t_sb = consts.tile([128, NCHUNK * H], F16)
            nc.sync.dma_start(out=w1t_sb, in_=w1t_d)
            w2t_sb = consts.tile([H, H], F16)
            nc.sync.dma_start(out=w2t_sb, in_=w2t_d)
            w3t_sb = consts.tile([H, O], F16)
            nc.sync.dma_start(out=w3t_sb, in_=w3t_d)
            negs_sb = consts.tile([1, H], F16)
            nc.sync.dma_start(out=negs_sb, in_=negs_d)
            b1_sb = consts.tile([H, 1], F32)
            nc.sync.dma_start(out=b1_sb, in_=b1_d)
            b2_sb = consts.tile([H, 1], F32)
            nc.sync.dma_start(out=b2_sb, in_=b2_d)
            b3_sb = consts.tile([O, 1], F32)
            nc.sync.dma_start(out=b3_sb, in_=b3_d)

            # ---- ACT table warm-up: force both activation table loads
            # while the engines are otherwise idle waiting for x ----
            warm = spool.tile([H, 1], F32, tag="warm")
            nc.scalar.activation(warm, b1_sb, AF.Abs_reciprocal_sqrt, scale=1.0)
            nc.scalar.activation(warm, b1_sb, AF.Prelu, bias=b1_sb, scale=1.0,
                                 alpha=0.01)
            nc.scalar.copy(warm, b1_sb)

            r0 = 0
            for nrows in BLOCKS:
                ns = nrows // 128          # sub-tiles in this block
                g = 1024 // nrows          # chunks per 1024-col PSUM group
                ngroups = NCHUNK // g

                # ---- load x block (fp32 -> fp16 cast in DMA) ----
                xs = []
                for s in range(ns):
                    xt = xpool.tile([128, D], F16, tag="xnat")
                    nc.gpsimd.dma_start(
                        out=xt, in_=x_d[r0 + s * 128:r0 + (s + 1) * 128, :]
                    )
                    xs.append(xt)

                # ---- per-row stats on DVE; one rsqrt per block on ACT ----
                mvall = spool.tile([128, 2, NSUB], F32, tag="mv")
                for s in range(ns):
                    st6 = spool.tile([128, 6, 6], F32, tag="st6")
                    for k in range(6):
                        nc.vector.bn_stats(
                            out=st6[:, k, :], in_=xs[s][:, k * 512:(k + 1) * 512]
                        )
                    nc.vector.bn_aggr(out=mvall[:, :, s], in_=st6)
                invall = spool.tile([128, NSUB], F32, tag="invc")
                nc.scalar.activation(invall[:, :ns], mvall[:, 1, :ns],
                                     AF.Abs_reciprocal_sqrt, scale=DDOF_SCALE)

                # ---- stats to row layout: [128,1] cols -> [1, nrows] rows ----
                pmean = pl_pool.tile([1, IBLK], F32, tag="pl")
                pinv = pl_pool.tile([1, IBLK], F32, tag="pl")
                for s in range(ns):
                    nc.tensor.transpose(
                        pmean[:, s * 128:(s + 1) * 128], mvall[:, 0, s:s + 1],
                        idf_sb
                    )
                    nc.tensor.transpose(
                        pinv[:, s * 128:(s + 1) * 128],
                        invall[:, s:s + 1], idf_sb
                    )
                mean_row = spool.tile([1, IBLK], F16, tag="mrow")
                nc.scalar.copy(mean_row[:, :nrows], pmean[0:1, :nrows])
                inv_row = spool.tile([1, IBLK], F16, tag="irow")
                nc.scalar.copy(inv_row[:, :nrows], pinv[0:1, :nrows])
                inv_b = spool.tile([H, IBLK], F16, tag="invb")
                nc.gpsimd.partition_broadcast(inv_b[:, :nrows],
                                              inv_row[:, :nrows])

                # ---- transpose x (as regular fp16 matmuls vs identity, to
                # keep the PE HAM-warm) + stream against w1t ----
                py0 = py0_pool.tile([H, IBLK], F32)
                prev = None
                for G in range(ngroups):
                    pxt = pxt_pool.tile([128, 1024], F32)
                    for j in range(g):
                        c = G * g + j
                        for s in range(ns):
                            nc.tensor.matmul(
                                pxt[:, j * nrows + s * 128:
                                    j * nrows + (s + 1) * 128],
                                xs[s][:, c * 128:(c + 1) * 128],
                                idh_sb,
                                start=True, stop=True,
                            )
                    xts = xtpool.tile([128, 1024], F16, tag="xt")
                    nc.scalar.copy(xts, pxt)
                    if prev is not None:
                        pG, pxts = prev
                        for j in range(g):
                            c = pG * g + j
                            nc.tensor.matmul(
                                py0[:, :nrows], w1t_sb[:, c * H:(c + 1) * H],
                                pxts[:, j * nrows:(j + 1) * nrows],
                                start=(c == 0), stop=False,
                            )
                    prev = (G, xts)
                pG, pxts = prev
                for j in range(g):
                    c = pG * g + j
                    nc.tensor.matmul(
                        py0[:, :nrows], w1t_sb[:, c * H:(c + 1) * H],
                        pxts[:, j * nrows:(j + 1) * nrows],
                        start=False, stop=False,
                    )
                # mean correction: y0 -= rowsum(w1) (x) mean  (K=1 matmul)
                nc.tensor.matmul(py0[:, :nrows], negs_sb, mean_row[:, :nrows],
                                 start=False, stop=True)

                # ---- normalize + layer 1 activation ----
                t1 = spool.tile([H, IBLK], F32, tag="t1")
                nc.vector.tensor_mul(t1[:, :nrows], py0[:, :nrows],
                                     inv_b[:, :nrows])
                h1 = spool.tile([H, IBLK], F16, tag="h1")
                nc.scalar.activation(h1[:, :nrows], t1[:, :nrows], AF.Prelu,
                                     bias=b1_sb, scale=1.0, alpha=0.01)

                # ---- layers 2 and 3 (small matmuls) ----
                p2 = pl_pool.tile([H, IBLK], F32, tag="pl")
                nc.tensor.matmul(p2[:, :nrows], w2t_sb, h1[:, :nrows],
                                 start=True, stop=True)
                h2 = spool.tile([H, IBLK], F16, tag="h2")
                nc.scalar.activation(h2[:, :nrows], p2[:, :nrows], AF.Prelu,
                                     bias=b2_sb, scale=1.0, alpha=0.01)
                p3 = pl_pool.tile([O, IBLK], F32, tag="pl")
                nc.tensor.matmul(p3[:, :nrows], w3t_sb, h2[:, :nrows],
                                 start=True, stop=True)
                y3 = spool.tile([O, IBLK], F32, tag="y3")
                nc.scalar.activation(y3[:, :nrows], p3[:, :nrows], AF.Prelu,
                                     bias=b3_sb, scale=1.0, alpha=0.01)

                # ---- store transposed; the host transposes back ----
                nc.sync.dma_start(
                    out=y_d[:, r0:r0 + nrows], in_=y3[:, :nrows],
                )
                r0 += nrows

    nc.compile()
    return nc


def _prep_inputs(x, w1, b1, w2, b2, w3, b3):
    x = np.ascontiguousarray(np.asarray(x, dtype=np.float32))
    w1 = np.asarray(w1, dtype=np.float32)
    w2 = np.asarray(w2, dtype=np.float32)
    w3 = np.asarray(w3, dtype=np.float32)
    b1 = np.asarray(b1, dtype=np.float32)
    b2 = np.asarray(b2, dtype=np.float32)
    b3 = np.asarray(b3, dtype=np.float32)

    common = {
        # [128, 24*32]: partition p holds w1.T[c*128+p, :] for each chunk c
        "w1t": np.ascontiguousarray(
            w1.T.reshape(NCHUNK, 128, H).transpose(1, 0, 2).reshape(128, NCHUNK * H)
        ).astype(np.float16),
        "w2t": np.ascontiguousarray(w2.T).astype(np.float16),
        "w3t": np.ascontiguousarray(w3.T).astype(np.float16),
        "negs": np.ascontiguousarray(
            -w1.astype(np.float64).sum(axis=1, keepdims=True).T
        ).astype(np.float16),
        "b1c": np.ascontiguousarray(b1[:, None]),
        "b2c": np.ascontiguousarray(b2[:, None]),
        "b3c": np.ascontiguousarray(b3[:, None]),
        "idh": np.eye(128, dtype=np.float16),
        "idf": np.eye(128, dtype=np.float32),
    }
    in_maps = []
    for c in range(N_CORES):
        m = dict(common)
        m["x"] = x[c * B_CORE:(c + 1) * B_CORE]
        in_maps.append(m)
    return in_maps


def kernel(x, w1, b1, w2, b2, w3, b3):
    global LAST_EXEC_NS
    if "nc" not in _CACHE:
        _CACHE["nc"] = _build()
    nc = _CACHE["nc"]
    in_maps = _prep_inputs(x, w1, b1, w2, b2, w3, b3)
    trace = bool(int(os.environ.get("KERNEL_PROFILE", "0")))
    res = run_bass_kernel_spmd(nc, in_maps, core_ids=list(range(N_CORES)),
                               trace=trace)
    LAST_EXEC_NS = res.exec_time_ns
    out = np.concatenate(
        [np.ascontiguousarray(r["y"].T) for r in res.results], axis=0
    )
    return out.astype(np.float32)
